# revision 64
# baseline (speedup 1.0000x reference)
"""Trainium2 Bass kernel for a dense transformer decoder block (B=4, T=2048,
C=1024, 16 heads x 64, DFF=4096), SPMD across 8 NeuronCores.

Sharding v2: parity token split. Core (b, p) owns the 8 odd/even 128-token
tiles of batch b (global tile g = 2*lt + p). The host permutes each core's
x to [own 8 tiles | peer 8 tiles] so the program is uniform. Causal
attention is balanced: query tile lt attends own key tiles j<=lt (diag
masked by tril) and peer key tiles m<=lt, where the m==lt block is fully
valid for p=1 and fully invalid for p=0 -- handled by a per-core 0/1
scalar mask input (pm). Rowsums come exactly from a ones-column appended
to V (no correction term needed).

A.V is computed transposed (V stationary, queries moving) so the result
lands as O^T [dims, tokens], feeding the Wo matmul directly with no PE
transposes. Scores run two heads concurrently in PE row groups 0-63 /
64-127 (K=64 row tiling). K/V projections for pair pr+1 are interleaved
with attention of pair pr so the scalar-engine exp stream hides under PE
work. All matmuls bf16 (fp32 PSUM); LN stats, softmax normalization and
residuals fp32. LN gamma/beta folded into adjacent weights on the host.
"""

import os
from contextlib import ExitStack

os.environ.setdefault("MYCRO_LOCAL_CACHE", "1")

import numpy as np
import ml_dtypes

import concourse.bacc as bacc
import concourse.bass as bass
import concourse.mybir as mybir
import concourse.tile as tile
from concourse.bass_utils import run_bass_kernel_spmd

BF16 = ml_dtypes.bfloat16
P = 128
C = 1024
H = 16
DH = 64
DFF = 4096
NPAIR = 8   # head pairs
NKT = 8     # C / 128 contraction tiles
NW = 16     # token tiles per batch sequence (2048 tokens)
NT = 8      # own token tiles (1024 tokens)
ND = 32     # DFF / 128 tiles
EPS = 1e-5

f32 = mybir.dt.float32
bf16 = mybir.dt.bfloat16
FT = mybir.ActivationFunctionType
ALU = mybir.AluOpType


def _attn_steps(qh):
    """Key steps for query half qh. Yields (kt_idx, is_peer, lt_min, q0, n).

    kt_idx: column tile in KT/VO (0..7 own keys, 8..15 peer keys).
    q0: first local query column covered; n: number of query columns.
    The first 128 query cols get the diagonal mask iff lt_min*128 >= qh*512.
    """
    out = []
    for peer in (0, 1):
        for j in range(qh * 4 + 4):
            q0 = max(j * P, qh * 512)
            n = (qh + 1) * 512 - q0
            out.append((peer * 8 + j, peer, j, q0, n))
    return out


def _build(flags):
    """Build the SPMD program. flags: dict of bools for nonzero biases."""
    nc = bacc.Bacc("TRN2", target_bir_lowering=False, debug=False, num_devices=8)

    # The attention phase interleaves Exp (softmax) with Ln (1/rowsum via
    # exp(-ln)). Both live in the natural_log_exp_and_others table set, but
    # the table-load placement maps Exp to exp_and_others (first match),
    # which would reload tables twice per head-pair. Steer Exp to the
    # combined set so the whole phase runs on one resident table.
    import concourse.hw_specs as hw_specs
    tabs = hw_specs.get_activation_tables(nc.m.arch)
    if "exp_and_others" in tabs and "natural_log_exp_and_others" in tabs:
        tabs["exp_and_others"].discard(mybir.ActivationFunctionType.Exp)

    xp = nc.dram_tensor("xp", [2048, C], f32, kind="ExternalInput")
    wq = nc.dram_tensor("wq", [P, 8192], bf16, kind="ExternalInput")
    wk = nc.dram_tensor("wk", [P, 8192], bf16, kind="ExternalInput")
    wv = nc.dram_tensor("wv", [P, 8192], bf16, kind="ExternalInput")
    wo = nc.dram_tensor("wo", [P, 8192], bf16, kind="ExternalInput")
    w1 = nc.dram_tensor("w1", [P, 32768], bf16, kind="ExternalInput")
    w2 = nc.dram_tensor("w2", [P, 32768], bf16, kind="ExternalInput")
    trilq = nc.dram_tensor("trilq", [P, P], bf16, kind="ExternalInput")
    identd = nc.dram_tensor("identd", [P, P], bf16, kind="ExternalInput")
    pmask = nc.dram_tensor("pmask", [P, 1], f32, kind="ExternalInput")
    qbias = nc.dram_tensor("qbias", [P, NPAIR], f32, kind="ExternalInput")
    kbias = nc.dram_tensor("kbias", [P, NPAIR], f32, kind="ExternalInput")
    b1p = nc.dram_tensor("b1p", [P, ND], f32, kind="ExternalInput")
    bo_row = nc.dram_tensor("bo_row", [P, C], f32, kind="ExternalInput")
    b2_row = nc.dram_tensor("b2_row", [P, C], f32, kind="ExternalInput")
    out = nc.dram_tensor("out", [1024, C], f32, kind="ExternalOutput")

    with tile.TileContext(nc) as tc, ExitStack() as es:
        consts = es.enter_context(tc.tile_pool(name="consts", bufs=1))
        tril_sb = consts.tile([P, P], bf16, tag="tril")
        nc.sync.dma_start(out=tril_sb[:, :], in_=trilq.ap()[:, :])
        ident_sb = consts.tile([P, P], bf16, tag="ident")
        nc.sync.dma_start(out=ident_sb[:, :], in_=identd.ap()[:, :])
        pm_sb = consts.tile([P, 1], f32, tag="pm")
        nc.sync.dma_start(out=pm_sb[:, :], in_=pmask.ap()[:, :])
        qb_sb = consts.tile([P, NPAIR], f32, tag="qb")
        nc.sync.dma_start(out=qb_sb[:, :], in_=qbias.ap()[:, :])
        kb_sb = consts.tile([P, NPAIR], f32, tag="kb")
        nc.sync.dma_start(out=kb_sb[:, :], in_=kbias.ap()[:, :])
        b1_sb = consts.tile([P, ND], f32, tag="b1")
        nc.sync.dma_start(out=b1_sb[:, :], in_=b1p.ap()[:, :])
        eps_sb = consts.tile([P, 1], f32, tag="eps")
        nc.vector.memset(eps_sb[:, :], EPS)
        # ones weights for the 1/rowsum broadcast matmul (partition 64)
        ones_sb = consts.tile([P, DH], bf16, tag="onesw")
        nc.vector.memset(ones_sb[:, :], 1.0)

        if flags["bo"]:
            bo_sb = consts.tile([P, C], f32, tag="bo")
            nc.sync.dma_start(out=bo_sb[:, :], in_=bo_row.ap()[:, :])
        if flags["b2"]:
            b2_sb = consts.tile([P, C], f32, tag="b2")
            nc.sync.dma_start(out=b2_sb[:, :], in_=b2_row.ap()[:, :])

        # persistent activation storage
        qt_pool = es.enter_context(tc.tile_pool(name="qt", bufs=NPAIR))
        kt_pool = es.enter_context(tc.tile_pool(name="kt", bufs=NPAIR))
        v_pool = es.enter_context(tc.tile_pool(name="vv", bufs=NW))
        x_pool = es.enter_context(tc.tile_pool(name="xx", bufs=NT))
        X = [x_pool.tile([P, C], f32, tag="xx", name=f"xt{i}") for i in range(NT)]
        ot_es = ExitStack()
        ot_pool = ot_es.enter_context(tc.tile_pool(name="oo", bufs=NPAIR))
        QT = [qt_pool.tile([P, 1024], bf16, tag="qt", name=f"qt{i}") for i in range(NPAIR)]
        KT = [kt_pool.tile([P, 2048], bf16, tag="kt", name=f"kt{i}") for i in range(NPAIR)]
        # V with interleaved ones columns: per pair 65+65 cols, keys = rows
        VO = [v_pool.tile([P, NPAIR * 130], bf16, tag="vv", name=f"vo{i}") for i in range(NW)]
        # O^T per pair: rows = C dims (head 2pr | head 2pr+1), cols = local q
        OT = [ot_pool.tile([P, 1024], bf16, tag="oo", name=f"ot{i}") for i in range(NPAIR)]

        def ln_tile(src_ap, lnp, zpool):
            """LayerNorm a [128, C] fp32 tile -> bf16 z tile (g/b folded out)."""
            if isinstance(src_ap, tuple):  # (dram_ap,) to load
                xw = lnp.tile([P, C], f32, tag="xw")
                nc.sync.dma_start(out=xw[:, :], in_=src_ap[0])
            else:
                xw = src_ap
            stats = lnp.tile([P, 2, 6], f32, tag="stats")
            nc.vector.bn_stats(out=stats[:, 0, :], in_=xw[:, 0:512])
            nc.vector.bn_stats(out=stats[:, 1, :], in_=xw[:, 512:1024])
            mv = lnp.tile([P, 2], f32, tag="mv")
            nc.vector.bn_aggr(out=mv[:, :], in_=stats[:, :, :])
            rsig = lnp.tile([P, 1], f32, tag="rsig")
            nc.scalar.activation(rsig[:, :], mv[:, 1:2], FT.Sqrt,
                                 bias=eps_sb[:, :], scale=1.0)
            nc.vector.reciprocal(rsig[:, :], rsig[:, :])
            z = zpool.tile([P, C], bf16, tag="z")
            nc.vector.tensor_scalar(z[:, :], xw[:, :], mv[:, 0:1], rsig[:, :],
                                    ALU.subtract, ALU.mult)
            return z

        def transpose_into(z, dst_tiles, wcol, tps):
            """PE-transpose [128, C] z into dst_tiles[c][:, wcol*128:+128]."""
            for c in range(NKT):
                tp = tps.tile([P, P], bf16, tag="tp")
                nc.tensor.transpose(tp[:, :], z[:, c * P:(c + 1) * P],
                                    ident_sb[:, :])
                nc.vector.tensor_copy(
                    out=dst_tiles[c][:, wcol * P:(wcol + 1) * P], in_=tp[:, :])

        # ---------------- Phase A: LN1, hT, V proj, Q projection --------------
        ht_es = ExitStack()
        ht_pool = ht_es.enter_context(tc.tile_pool(name="ht", bufs=NKT))
        wv_pool = ht_es.enter_context(tc.tile_pool(name="wvp", bufs=NKT))
        HT = [ht_pool.tile([P, 2048], bf16, tag="ht", name=f"ht{i}") for i in range(NKT)]
        WV = [wv_pool.tile([P, 1024], bf16, tag="wv", name=f"wvt{i}")
              for i in range(NKT)]
        for kt in range(NKT):
            nc.sync.dma_start(out=WV[kt][:, :],
                              in_=wv.ap()[:, kt * 1024:(kt + 1) * 1024])

        def vgroup(w, hf, pool=None):
            """Project V of pairs hf*4..hf*4+3 for token tile w."""
            pv = (pool or vps).tile([P, 512], f32, tag="vv")
            for kt in range(NKT):
                nc.tensor.matmul(
                    pv[:, :], HT[kt][:, w * P:(w + 1) * P],
                    WV[kt][:, hf * 512:(hf + 1) * 512],
                    start=(kt == 0), stop=(kt == NKT - 1))
            blk = VO[w][:, hf * 4 * 130:(hf + 1) * 4 * 130]
            vdst = blk.rearrange("p (pr hi dd) -> p pr hi dd",
                                 pr=4, hi=2)[:, :, :, 0:64]
            vsrc = pv[:, :].rearrange("p (pr hi dd) -> p pr hi dd",
                                      pr=4, hi=2)
            nc.vector.tensor_copy(out=vdst, in_=vsrc)
            ones = blk.rearrange("p (pr hi dd) -> p pr hi dd",
                                 pr=4, hi=2)[:, :, :, 64:65]
            nc.vector.memset(ones, 1.0)

        with tc.tile_pool(name="ln1", bufs=3) as lnp, \
             tc.tile_pool(name="z1", bufs=3) as zpool, \
             tc.tile_pool(name="tps1", bufs=2, space="PSUM") as tps1, \
             tc.tile_pool(name="wqp", bufs=2) as wq_pool, \
             tc.tile_pool(name="vps", bufs=2, space="PSUM") as vps, \
             tc.tile_pool(name="qps", bufs=2, space="PSUM") as qps:

            def qproj(pr):
                wq_sb = wq_pool.tile([P, 1024], bf16, tag="wq",
                                     name=f"wq{pr}")
                nc.sync.dma_start(out=wq_sb[:, :],
                                  in_=wq.ap()[:, pr * 1024:(pr + 1) * 1024])
                for qh in range(2):
                    pq = qps.tile([P, 512], f32, tag="qps")
                    for kt in range(NKT):
                        nc.tensor.matmul(
                            pq[:, :], wq_sb[:, kt * P:(kt + 1) * P],
                            HT[kt][:, qh * 512:(qh + 1) * 512],
                            start=(kt == 0), stop=(kt == NKT - 1))
                    # (q + qb) * 1/sqrt(dh)
                    nc.vector.tensor_scalar(
                        QT[pr][:, qh * 512:(qh + 1) * 512], pq[:, :],
                        qb_sb[:, pr:pr + 1], 0.125, ALU.add, ALU.mult)

            for w in range(NW):
                z = ln_tile((xp.ap()[w * P:(w + 1) * P, :],), lnp, zpool)
                transpose_into(z, HT, w, tps1)
                vgroup(w, 0)
                vgroup(w, 1)
                if w >= NT:
                    qproj(w - NT)

        # ---------------- Phase B: K/V proj pipelined with attention ----------
        with tc.tile_pool(name="wkp", bufs=2) as wk_pool, \
             tc.tile_pool(name="kps", bufs=1, space="PSUM") as kps, \
             tc.tile_pool(name="sps", bufs=2, space="PSUM") as sps, \
             tc.tile_pool(name="avps", bufs=3, space="PSUM") as avps, \
             tc.tile_pool(name="ep", bufs=3) as ep_pool, \
             tc.tile_pool(name="rsn", bufs=2) as rs_pool, \
             tc.tile_pool(name="osc", bufs=2) as osc_pool:

            def kgroup(pr, wk_sb, wh):
                pk = kps.tile([P, 512], f32, tag="vv")
                for kt in range(NKT):
                    nc.tensor.matmul(
                        pk[:, :], wk_sb[:, kt * P:(kt + 1) * P],
                        HT[kt][:, wh * 512:(wh + 1) * 512],
                        start=(kt == 0), stop=(kt == NKT - 1))
                nc.vector.tensor_scalar_add(
                    KT[pr][:, wh * 512:(wh + 1) * 512], pk[:, :],
                    kb_sb[:, pr:pr + 1])

            def kproj_groups(pr):
                wk_sb = wk_pool.tile([P, 1024], bf16, tag="wk",
                                     name=f"wk{pr}")
                nc.sync.dma_start(out=wk_sb[:, :],
                                  in_=wk.ap()[:, pr * 1024:(pr + 1) * 1024])
                return [lambda wh=wh: kgroup(pr, wk_sb, wh)
                        for wh in range(4)]

            feeds = [[] for _ in range(NPAIR)]
            prologue = kproj_groups(0)
            for pr in range(NPAIR - 1):
                feeds[pr] = kproj_groups(pr + 1)

            pending = [None]

            def attention(pr, feed):
                """Attention for pair pr; calls next(feed) between steps."""
                for qh in range(2):
                    av = [avps.tile([65, 512], f32, tag="av", name=f"av{hi}")
                          for hi in range(2)]
                    steps = _attn_steps(qh)
                    for si, (kt_idx, peer, lt_min, q0, n) in enumerate(steps):
                        next(feed, None)
                        # scores: both heads concurrent via PE row groups;
                        # head chunks at fixed 512-col (bank) offsets
                        sp = sps.tile([P, 1024], f32, tag="sps")
                        for hi in range(2):
                            nc.tensor.matmul(
                                sp[:, hi * 512:hi * 512 + n],
                                KT[pr][hi * 64:(hi + 1) * 64,
                                       kt_idx * P:(kt_idx + 1) * P],
                                QT[pr][hi * 64:(hi + 1) * 64, q0:q0 + n],
                                start=True, stop=True)
                        ep = ep_pool.tile([P, 1024], bf16, tag="ep")
                        if n == 512:
                            # per-head halves: A.V of head 0 starts earlier
                            nc.scalar.activation(ep[:, 0:512], sp[:, 0:512],
                                                 FT.Exp)
                            nc.scalar.activation(ep[:, 512:1024],
                                                 sp[:, 512:1024], FT.Exp)
                        else:
                            sp3 = sp[:, :].rearrange(
                                "p (hi q) -> p hi q", hi=2)[:, :, 0:n]
                            ep3 = ep[:, :].rearrange(
                                "p (hi q) -> p hi q", hi=2)[:, :, 0:n]
                            nc.scalar.activation(ep3, sp3, FT.Exp)
                        if lt_min * P >= qh * 512:
                            # diagonal block: first 128 query cols of each head
                            for hi in range(2):
                                sl = ep[:, hi * 512:hi * 512 + P]
                                if peer:
                                    nc.vector.tensor_scalar_mul(
                                        sl, sl, pm_sb[:, 0:1])
                                else:
                                    nc.vector.tensor_mul(sl, sl, tril_sb[:, :])
                        first, last = si == 0, si == len(steps) - 1
                        for hi in range(2):
                            nc.tensor.matmul(
                                av[hi][:, q0 - qh * 512:q0 - qh * 512 + n],
                                VO[kt_idx][:, pr * 130 + hi * 65:
                                           pr * 130 + hi * 65 + 65],
                                ep[:, hi * 512:hi * 512 + n],
                                start=first, stop=last)
                    # normalize: O^T[d, q] = av[d, q] / av[64, q]. 1/rowsum =
                    # exp(-ln(rs)) on the scalar engine (Ln and Exp share the
                    # natural_log table set), computed on the partition-64
                    # row, then broadcast to 64 partitions by a K=1 bf16
                    # matmul and staged to SBUF for the DVE multiply
                    next(feed, None)
                    bcs = []
                    for hi in range(2):
                        lnr = rs_pool.tile([65, 512], f32, tag="rs",
                                           name=f"lnr{hi}")
                        nc.scalar.activation(lnr[64:65, :],
                                             av[hi][64:65, :], FT.Ln)
                        rrow = rs_pool.tile([65, 512], bf16, tag="rrow",
                                            name=f"rrow{hi}")
                        nc.scalar.activation(rrow[64:65, :],
                                             lnr[64:65, :], FT.Exp,
                                             scale=-1.0)
                        bc = sps.tile([64, 512], f32, tag="sps",
                                      name=f"bc{hi}")
                        nc.tensor.matmul(bc[:, :], ones_sb[64:65, :],
                                         rrow[64:65, :], start=True, stop=True)
                        bch = rs_pool.tile([64, 512], bf16, tag="bcs",
                                           name=f"bch{hi}")
                        nc.vector.tensor_copy(out=bch[:, :], in_=bc[:, :])
                        bcs.append(bch)
                    # head 0 writes OT rows 0:64 directly; head 1 goes via
                    # an SBUF scratch + DMA partition shift to rows 64:128
                    nc.vector.tensor_mul(OT[pr][0:64, qh * 512:(qh + 1) * 512],
                                         av[0][0:64, :], bcs[0][:, :])
                    osc = osc_pool.tile([64, 512], bf16, tag="osc")
                    nc.vector.tensor_mul(osc[:, :], av[1][0:64, :],
                                         bcs[1][:, :])
                    nc.sync.dma_start(
                        out=OT[pr][64:128, qh * 512:(qh + 1) * 512],
                        in_=osc[:, :])

            for g in prologue:
                g()
            # residual inputs: prefetch during attention
            for it in range(NT):
                nc.sync.dma_start(out=X[it][:, :],
                                  in_=xp.ap()[it * P:(it + 1) * P, :])
            for pr in range(NPAIR):
                fl = iter(feeds[pr])
                attention(pr, (g() for g in fl))
                for g in fl:
                    g()
        ht_es.close()

        # ---------------- Phase C: Wo, residual, LN2, FFN, store --------------
        with tc.tile_pool(name="wos", bufs=1) as wo_pool, \
             tc.tile_pool(name="wops", bufs=2, space="PSUM") as wops:
            wo_sb = wo_pool.tile([P, 8192], bf16, tag="wo")
            nc.sync.dma_start(out=wo_sb[:, :], in_=wo.ap()[:, :])
            for it in range(NT):
                pw = wops.tile([P, 1024], f32, tag="wops")
                for kt in range(NKT):
                    for hf in range(2):
                        nc.tensor.matmul(
                            pw[:, hf * 512:(hf + 1) * 512],
                            OT[kt][:, it * P:(it + 1) * P],
                            wo_sb[:, kt * 1024 + hf * 512:
                                  kt * 1024 + (hf + 1) * 512],
                            start=(kt == 0), stop=(kt == NKT - 1))
                nc.vector.tensor_add(X[it][:, :], pw[:, :], X[it][:, :])
                if flags["bo"]:
                    nc.vector.tensor_add(X[it][:, :], X[it][:, :], bo_sb[:, :])
        ot_es.close()

        with tc.tile_pool(name="ln2", bufs=3) as lnp2, \
             tc.tile_pool(name="z2", bufs=3) as zpool2, \
             tc.tile_pool(name="h2t", bufs=NKT) as h2t_pool, \
             tc.tile_pool(name="tps3", bufs=2, space="PSUM") as tps3, \
             tc.tile_pool(name="ut", bufs=ND) as ut_pool, \
             tc.tile_pool(name="w1s", bufs=8) as w1_pool, \
             tc.tile_pool(name="w2s", bufs=8) as w2_pool, \
             tc.tile_pool(name="ups", bufs=2, space="PSUM") as ups, \
             tc.tile_pool(name="yps", bufs=4, space="PSUM") as yps:
            H2T = [h2t_pool.tile([P, 1024], bf16, tag="h2t", name=f"h2t{i}") for i in range(NKT)]
            for it in range(NT):
                z2 = ln_tile(X[it], lnp2, zpool2)
                transpose_into(z2, H2T, it, tps3)
            for tch in range(2):
                UT = [ut_pool.tile([P, 512], bf16, tag="ut", name=f"ut{i}") for i in range(ND)]
                for d in range(ND):
                    w1_sb = w1_pool.tile([P, 1024], bf16, tag="w1")
                    nc.sync.dma_start(
                        out=w1_sb[:, :],
                        in_=w1.ap()[:, d * 1024:(d + 1) * 1024])
                    pu = ups.tile([P, 512], f32, tag="ups")
                    for kt in range(NKT):
                        nc.tensor.matmul(
                            pu[:, :], w1_sb[:, kt * P:(kt + 1) * P],
                            H2T[kt][:, tch * 512:(tch + 1) * 512],
                            start=(kt == 0), stop=(kt == NKT - 1))
                    # relu(x + b1)
                    nc.vector.tensor_scalar(UT[d][:, :], pu[:, :],
                                            b1_sb[:, d:d + 1], 0.0,
                                            ALU.add, ALU.max)
                for ch in range(2):
                    ypsum = [yps.tile([P, 512], f32, tag="yps", name=f"yps{i}")
                             for i in range(4)]
                    for d in range(ND):
                        w2_sb = w2_pool.tile([P, 512], bf16, tag="w2")
                        nc.sync.dma_start(
                            out=w2_sb[:, :],
                            in_=w2.ap()[:, d * 1024 + ch * 512:
                                        d * 1024 + (ch + 1) * 512])
                        for tt in range(4):
                            nc.tensor.matmul(
                                ypsum[tt][:, :],
                                UT[d][:, tt * P:(tt + 1) * P],
                                w2_sb[:, :],
                                start=(d == 0), stop=(d == ND - 1))
                    for tt in range(4):
                        it = tch * 4 + tt
                        xsl = X[it][:, ch * 512:(ch + 1) * 512]
                        nc.vector.tensor_add(xsl, ypsum[tt][:, :], xsl)
                        if flags["b2"]:
                            nc.vector.tensor_add(
                                xsl, xsl, b2_sb[:, ch * 512:(ch + 1) * 512])
            for it in range(NT):
                nc.sync.dma_start(out=out.ap()[it * P:(it + 1) * P, :],
                                  in_=X[it][:, :])

    nc.compile()
    return nc


_CACHE = {}


def _prep(inputs):
    """Host-side preprocessing: fold LN affine into weights, tile/cast, shard."""
    x = np.asarray(inputs["x"], np.float32)
    Wq = np.asarray(inputs["Wq"], np.float32)
    Wk = np.asarray(inputs["Wk"], np.float32)
    Wv = np.asarray(inputs["Wv"], np.float32)
    Wo = np.asarray(inputs["Wo"], np.float32)
    bo = np.asarray(inputs["bo"], np.float32)
    W1 = np.asarray(inputs["W1"], np.float32)
    b1 = np.asarray(inputs["b1"], np.float32)
    W2 = np.asarray(inputs["W2"], np.float32)
    b2 = np.asarray(inputs["b2"], np.float32)
    g1 = np.asarray(inputs["g1"], np.float32)
    be1 = np.asarray(inputs["be1"], np.float32)
    g2 = np.asarray(inputs["g2"], np.float32)
    be2 = np.asarray(inputs["be2"], np.float32)

    Wq_g = (Wq * g1[None, :, None]).astype(BF16)   # [16,1024,64]
    Wk_g = (Wk * g1[None, :, None]).astype(BF16)
    Wv_g = (Wv * g1[None, :, None]).astype(BF16)
    qb = np.einsum('c,hcd->hd', be1, Wq_g.astype(np.float32))  # [16,64]
    kb = np.einsum('c,hcd->hd', be1, Wk_g.astype(np.float32))
    vb = np.einsum('c,hcd->hd', be1, Wv_g.astype(np.float32))
    if np.abs(vb).max() > 0:
        raise NotImplementedError("nonzero folded V bias not supported")

    def lhsT_pack(wflat):  # [1024 c, 1024 m] -> [128, (pair, kt, 128)]
        return np.ascontiguousarray(
            wflat.reshape(8, 128, 8, 128).transpose(1, 2, 0, 3).reshape(128, 8192))

    def rhs_pack(wflat):   # [1024 k, 1024 n] -> [128, (kt, 1024)]
        return np.ascontiguousarray(
            wflat.reshape(8, 128, 1024).transpose(1, 0, 2).reshape(128, 8192))

    wq_h = lhsT_pack(Wq_g.transpose(1, 0, 2).reshape(1024, 1024))
    wk_h = lhsT_pack(Wk_g.transpose(1, 0, 2).reshape(1024, 1024))
    wv_h = rhs_pack(Wv_g.transpose(1, 0, 2).reshape(1024, 1024))
    wo_h = rhs_pack(Wo.astype(BF16))
    W1_g = (W1 * g2[:, None]).astype(BF16)         # [1024, 4096]
    b1p = b1 + be2 @ W1_g.astype(np.float32)
    w1_h = np.ascontiguousarray(
        W1_g.reshape(8, 128, 32, 128).transpose(1, 2, 0, 3).reshape(128, 32768))
    w2_h = np.ascontiguousarray(
        W2.astype(BF16).reshape(32, 128, 1024).transpose(1, 0, 2).reshape(128, 32768))

    # per-pair stacked [128, 8] bias tables
    qb_t = np.zeros((128, 8), np.float32)
    kb_t = np.zeros((128, 8), np.float32)
    for pr in range(8):
        qb_t[0:64, pr] = qb[2 * pr]
        qb_t[64:128, pr] = qb[2 * pr + 1]
        kb_t[0:64, pr] = kb[2 * pr]
        kb_t[64:128, pr] = kb[2 * pr + 1]
    b1_t = np.ascontiguousarray(b1p.reshape(32, 128).T.astype(np.float32))
    bo_t = np.broadcast_to(bo, (128, 1024)).astype(np.float32).copy()
    b2_t = np.broadcast_to(b2, (128, 1024)).astype(np.float32).copy()

    tril = np.triu(np.ones((128, 128), np.float32)).astype(BF16)
    ident = np.eye(128, dtype=np.float32).astype(BF16)

    flags = {"bo": bool(np.abs(bo).max() > 0), "b2": bool(np.abs(b2).max() > 0)}

    shared = dict(wq=wq_h, wk=wk_h, wv=wv_h, wo=wo_h, w1=w1_h, w2=w2_h,
                  trilq=tril, identd=ident, qbias=qb_t, kbias=kb_t,
                  b1p=b1_t, bo_row=bo_t, b2_row=b2_t)
    in_maps = []
    xt = x.reshape(4, 16, 128, 1024)
    for core in range(8):
        b, p = core // 2, core % 2
        own = xt[b, p::2]                  # [8, 128, 1024] global tiles 2lt+p
        peer = xt[b, 1 - p::2]             # [8, 128, 1024] global tiles 2m+1-p
        xperm = np.concatenate([own, peer], 0).reshape(2048, 1024)
        pm = np.full((128, 1), 1.0 if p == 1 else 0.0, np.float32)
        in_maps.append({"xp": np.ascontiguousarray(xperm), "pmask": pm,
                        **shared})
    return in_maps, flags


def _get_nc(flags):
    key = tuple(sorted(flags.items()))
    if key not in _CACHE:
        _CACHE[key] = _build(flags)
    return _CACHE[key]


def run(inputs, **kw):
    in_maps, flags = _prep(inputs)
    nc = _get_nc(flags)
    res = run_bass_kernel_spmd(nc, in_maps, core_ids=list(range(8)), **kw)
    x = np.asarray(inputs["x"], np.float32)
    outf = np.zeros_like(x)
    for core in range(8):
        b, p = core // 2, core % 2
        o = res.results[core]["out"].reshape(8, 128, 1024)
        outf[b].reshape(16, 128, 1024)[p::2] = o
    return outf, res


def kernel(**inputs):
    outf, _ = run(inputs)
    return outf


# revision 65
# speedup vs baseline: 1.0454x; 1.0454x over previous
"""Trainium2 Bass kernel for a dense transformer decoder block (B=4, T=2048,
C=1024, 16 heads x 64, DFF=4096), SPMD across 8 NeuronCores.

Sharding v2: parity token split. Core (b, p) owns the 8 odd/even 128-token
tiles of batch b (global tile g = 2*lt + p). The host permutes each core's
x to [own 8 tiles | peer 8 tiles] so the program is uniform. Causal
attention is balanced: query tile lt attends own key tiles j<=lt (diag
masked by tril) and peer key tiles m<=lt, where the m==lt block is fully
valid for p=1 and fully invalid for p=0 -- handled by a per-core 0/1
scalar mask input (pm). Rowsums come exactly from a ones-column appended
to V (no correction term needed).

A.V is computed transposed (V stationary, queries moving) so the result
lands as O^T [dims, tokens], feeding the Wo matmul directly with no PE
transposes. Scores run two heads concurrently in PE row groups 0-63 /
64-127 (K=64 row tiling). K/V projections for pair pr+1 are interleaved
with attention of pair pr so the scalar-engine exp stream hides under PE
work. All matmuls bf16 (fp32 PSUM); LN stats, softmax normalization and
residuals fp32. LN gamma/beta folded into adjacent weights on the host.
"""

import os
from contextlib import ExitStack

os.environ.setdefault("MYCRO_LOCAL_CACHE", "1")

import numpy as np
import ml_dtypes

import concourse.bacc as bacc
import concourse.bass as bass
import concourse.mybir as mybir
import concourse.tile as tile
from concourse.bass_utils import run_bass_kernel_spmd

BF16 = ml_dtypes.bfloat16
P = 128
C = 1024
H = 16
DH = 64
DFF = 4096
NPAIR = 8   # head pairs
NKT = 8     # C / 128 contraction tiles
NW = 16     # token tiles per batch sequence (2048 tokens)
NT = 8      # own token tiles (1024 tokens)
ND = 32     # DFF / 128 tiles
EPS = 1e-5

f32 = mybir.dt.float32
bf16 = mybir.dt.bfloat16
FT = mybir.ActivationFunctionType
ALU = mybir.AluOpType


def _attn_steps(qh):
    """Key steps for query half qh. Yields (kt_idx, is_peer, lt_min, q0, n).

    kt_idx: column tile in KT/VO (0..7 own keys, 8..15 peer keys).
    q0: first local query column covered; n: number of query columns.
    The first 128 query cols get the diagonal mask iff lt_min*128 >= qh*512.
    """
    out = []
    for peer in (0, 1):
        for j in range(qh * 4 + 4):
            q0 = max(j * P, qh * 512)
            n = (qh + 1) * 512 - q0
            out.append((peer * 8 + j, peer, j, q0, n))
    return out


def _build(flags):
    """Build the SPMD program. flags: dict of bools for nonzero biases."""
    nc = bacc.Bacc("TRN2", target_bir_lowering=False, debug=False, num_devices=8)

    # The attention phase interleaves Exp (softmax) with Ln (1/rowsum via
    # exp(-ln)). Both live in the natural_log_exp_and_others table set, but
    # the table-load placement maps Exp to exp_and_others (first match),
    # which would reload tables twice per head-pair. Steer Exp to the
    # combined set so the whole phase runs on one resident table.
    import concourse.hw_specs as hw_specs
    tabs = hw_specs.get_activation_tables(nc.m.arch)
    if "exp_and_others" in tabs and "natural_log_exp_and_others" in tabs:
        tabs["exp_and_others"].discard(mybir.ActivationFunctionType.Exp)

    xp = nc.dram_tensor("xp", [2048, C], f32, kind="ExternalInput")
    wq = nc.dram_tensor("wq", [P, 8192], bf16, kind="ExternalInput")
    wk = nc.dram_tensor("wk", [P, 8192], bf16, kind="ExternalInput")
    wv = nc.dram_tensor("wv", [P, 8192], bf16, kind="ExternalInput")
    wo = nc.dram_tensor("wo", [P, 8192], bf16, kind="ExternalInput")
    w1 = nc.dram_tensor("w1", [P, 32768], bf16, kind="ExternalInput")
    w2 = nc.dram_tensor("w2", [P, 32768], bf16, kind="ExternalInput")
    trilq = nc.dram_tensor("trilq", [P, P], bf16, kind="ExternalInput")
    identd = nc.dram_tensor("identd", [P, P], bf16, kind="ExternalInput")
    pmask = nc.dram_tensor("pmask", [P, 1], f32, kind="ExternalInput")
    qbias = nc.dram_tensor("qbias", [P, NPAIR], f32, kind="ExternalInput")
    kbias = nc.dram_tensor("kbias", [P, NPAIR], f32, kind="ExternalInput")
    b1p = nc.dram_tensor("b1p", [P, ND], f32, kind="ExternalInput")
    bo_row = nc.dram_tensor("bo_row", [P, C], f32, kind="ExternalInput")
    b2_row = nc.dram_tensor("b2_row", [P, C], f32, kind="ExternalInput")
    out = nc.dram_tensor("out", [1024, C], f32, kind="ExternalOutput")

    with tile.TileContext(nc) as tc, ExitStack() as es:
        consts = es.enter_context(tc.tile_pool(name="consts", bufs=1))
        tril_sb = consts.tile([P, P], bf16, tag="tril")
        nc.sync.dma_start(out=tril_sb[:, :], in_=trilq.ap()[:, :])
        ident_sb = consts.tile([P, P], bf16, tag="ident")
        nc.sync.dma_start(out=ident_sb[:, :], in_=identd.ap()[:, :])
        pm_sb = consts.tile([P, 1], f32, tag="pm")
        nc.sync.dma_start(out=pm_sb[:, :], in_=pmask.ap()[:, :])
        qb_sb = consts.tile([P, NPAIR], f32, tag="qb")
        nc.sync.dma_start(out=qb_sb[:, :], in_=qbias.ap()[:, :])
        kb_sb = consts.tile([P, NPAIR], f32, tag="kb")
        nc.sync.dma_start(out=kb_sb[:, :], in_=kbias.ap()[:, :])
        b1_sb = consts.tile([P, ND], f32, tag="b1")
        nc.sync.dma_start(out=b1_sb[:, :], in_=b1p.ap()[:, :])
        eps_sb = consts.tile([P, 1], f32, tag="eps")
        nc.vector.memset(eps_sb[:, :], EPS)
        # ones weights for the 1/rowsum broadcast matmul (partition 64)
        ones_sb = consts.tile([P, DH], bf16, tag="onesw")
        nc.vector.memset(ones_sb[:, :], 1.0)

        if flags["bo"]:
            bo_sb = consts.tile([P, C], f32, tag="bo")
            nc.sync.dma_start(out=bo_sb[:, :], in_=bo_row.ap()[:, :])
        if flags["b2"]:
            b2_sb = consts.tile([P, C], f32, tag="b2")
            nc.sync.dma_start(out=b2_sb[:, :], in_=b2_row.ap()[:, :])

        # persistent activation storage
        qt_pool = es.enter_context(tc.tile_pool(name="qt", bufs=NPAIR))
        kt_pool = es.enter_context(tc.tile_pool(name="kt", bufs=NPAIR))
        v_pool = es.enter_context(tc.tile_pool(name="vv", bufs=NW))
        x_pool = es.enter_context(tc.tile_pool(name="xx", bufs=NT))
        X = [x_pool.tile([P, C], f32, tag="xx", name=f"xt{i}") for i in range(NT)]
        ot_es = ExitStack()
        ot_pool = ot_es.enter_context(tc.tile_pool(name="oo", bufs=NPAIR))
        QT = [qt_pool.tile([P, 1024], bf16, tag="qt", name=f"qt{i}") for i in range(NPAIR)]
        KT = [kt_pool.tile([P, 2048], bf16, tag="kt", name=f"kt{i}") for i in range(NPAIR)]
        # V with interleaved ones columns: per pair 65+65 cols, keys = rows
        VO = [v_pool.tile([P, NPAIR * 130], bf16, tag="vv", name=f"vo{i}") for i in range(NW)]
        # O^T per pair: rows = C dims (head 2pr | head 2pr+1), cols = local q
        OT = [ot_pool.tile([P, 1024], bf16, tag="oo", name=f"ot{i}") for i in range(NPAIR)]

        def ln_tile(src_ap, lnp, zpool):
            """LayerNorm a [128, C] fp32 tile -> bf16 z tile (g/b folded out)."""
            if isinstance(src_ap, tuple):  # (dram_ap,) to load
                xw = lnp.tile([P, C], f32, tag="xw")
                nc.sync.dma_start(out=xw[:, :], in_=src_ap[0])
            else:
                xw = src_ap
            stats = lnp.tile([P, 2, 6], f32, tag="stats")
            nc.vector.bn_stats(out=stats[:, 0, :], in_=xw[:, 0:512])
            nc.vector.bn_stats(out=stats[:, 1, :], in_=xw[:, 512:1024])
            mv = lnp.tile([P, 2], f32, tag="mv")
            nc.vector.bn_aggr(out=mv[:, :], in_=stats[:, :, :])
            rsig = lnp.tile([P, 1], f32, tag="rsig")
            nc.scalar.activation(rsig[:, :], mv[:, 1:2], FT.Sqrt,
                                 bias=eps_sb[:, :], scale=1.0)
            nc.vector.reciprocal(rsig[:, :], rsig[:, :])
            z = zpool.tile([P, C], bf16, tag="z")
            nc.vector.tensor_scalar(z[:, :], xw[:, :], mv[:, 0:1], rsig[:, :],
                                    ALU.subtract, ALU.mult)
            return z

        def transpose_into(z, dst_tiles, wcol, tps):
            """PE-transpose [128, C] z into dst_tiles[c][:, wcol*128:+128]."""
            for c in range(NKT):
                tp = tps.tile([P, P], bf16, tag="tp")
                nc.tensor.transpose(tp[:, :], z[:, c * P:(c + 1) * P],
                                    ident_sb[:, :])
                nc.vector.tensor_copy(
                    out=dst_tiles[c][:, wcol * P:(wcol + 1) * P], in_=tp[:, :])

        # ---------------- Phase A: LN1, hT, V proj, Q projection --------------
        ht_es = ExitStack()
        ht_pool = ht_es.enter_context(tc.tile_pool(name="ht", bufs=NKT))
        wv_pool = ht_es.enter_context(tc.tile_pool(name="wvp", bufs=NKT))
        HT = [ht_pool.tile([P, 2048], bf16, tag="ht", name=f"ht{i}") for i in range(NKT)]
        WV = [wv_pool.tile([P, 1024], bf16, tag="wv", name=f"wvt{i}")
              for i in range(NKT)]
        for kt in range(NKT):
            nc.sync.dma_start(out=WV[kt][:, :],
                              in_=wv.ap()[:, kt * 1024:(kt + 1) * 1024])

        def vgroup(w, hf, pool=None):
            """Project V of pairs hf*4..hf*4+3 for token tile w."""
            pv = (pool or vps).tile([P, 512], f32, tag="vv")
            for kt in range(NKT):
                nc.tensor.matmul(
                    pv[:, :], HT[kt][:, w * P:(w + 1) * P],
                    WV[kt][:, hf * 512:(hf + 1) * 512],
                    start=(kt == 0), stop=(kt == NKT - 1))
            blk = VO[w][:, hf * 4 * 130:(hf + 1) * 4 * 130]
            vdst = blk.rearrange("p (pr hi dd) -> p pr hi dd",
                                 pr=4, hi=2)[:, :, :, 0:64]
            vsrc = pv[:, :].rearrange("p (pr hi dd) -> p pr hi dd",
                                      pr=4, hi=2)
            nc.vector.tensor_copy(out=vdst, in_=vsrc)
            ones = blk.rearrange("p (pr hi dd) -> p pr hi dd",
                                 pr=4, hi=2)[:, :, :, 64:65]
            nc.vector.memset(ones, 1.0)

        with tc.tile_pool(name="ln1", bufs=3) as lnp, \
             tc.tile_pool(name="z1", bufs=3) as zpool, \
             tc.tile_pool(name="tps1", bufs=2, space="PSUM") as tps1, \
             tc.tile_pool(name="wqp", bufs=2) as wq_pool, \
             tc.tile_pool(name="vps", bufs=2, space="PSUM") as vps, \
             tc.tile_pool(name="qps", bufs=2, space="PSUM") as qps:

            def qproj(pr):
                wq_sb = wq_pool.tile([P, 1024], bf16, tag="wq",
                                     name=f"wq{pr}")
                nc.sync.dma_start(out=wq_sb[:, :],
                                  in_=wq.ap()[:, pr * 1024:(pr + 1) * 1024])
                for qh in range(2):
                    pq = qps.tile([P, 512], f32, tag="qps")
                    for kt in range(NKT):
                        nc.tensor.matmul(
                            pq[:, :], wq_sb[:, kt * P:(kt + 1) * P],
                            HT[kt][:, qh * 512:(qh + 1) * 512],
                            start=(kt == 0), stop=(kt == NKT - 1))
                    # (q + qb) * 1/sqrt(dh)
                    nc.vector.tensor_scalar(
                        QT[pr][:, qh * 512:(qh + 1) * 512], pq[:, :],
                        qb_sb[:, pr:pr + 1], 0.125, ALU.add, ALU.mult)

            for w in range(NW):
                z = ln_tile((xp.ap()[w * P:(w + 1) * P, :],), lnp, zpool)
                transpose_into(z, HT, w, tps1)
                vgroup(w, 0)
                vgroup(w, 1)
                if w >= NT:
                    qproj(w - NT)

        # ---------------- Phase B: K/V proj pipelined with attention ----------
        with tc.tile_pool(name="wkp", bufs=2) as wk_pool, \
             tc.tile_pool(name="kps", bufs=1, space="PSUM") as kps, \
             tc.tile_pool(name="sps", bufs=2, space="PSUM") as sps, \
             tc.tile_pool(name="avps", bufs=3, space="PSUM") as avps, \
             tc.tile_pool(name="ep", bufs=3) as ep_pool, \
             tc.tile_pool(name="rsn", bufs=2) as rs_pool, \
             tc.tile_pool(name="osc", bufs=2) as osc_pool:

            def kgroup(pr, wk_sb, wh):
                pk = kps.tile([P, 512], f32, tag="vv")
                for kt in range(NKT):
                    nc.tensor.matmul(
                        pk[:, :], wk_sb[:, kt * P:(kt + 1) * P],
                        HT[kt][:, wh * 512:(wh + 1) * 512],
                        start=(kt == 0), stop=(kt == NKT - 1))
                nc.vector.tensor_scalar_add(
                    KT[pr][:, wh * 512:(wh + 1) * 512], pk[:, :],
                    kb_sb[:, pr:pr + 1])

            def kproj_groups(pr):
                wk_sb = wk_pool.tile([P, 1024], bf16, tag="wk",
                                     name=f"wk{pr}")
                nc.sync.dma_start(out=wk_sb[:, :],
                                  in_=wk.ap()[:, pr * 1024:(pr + 1) * 1024])
                return [lambda wh=wh: kgroup(pr, wk_sb, wh)
                        for wh in range(4)]

            feeds = [[] for _ in range(NPAIR)]
            prologue = kproj_groups(0)
            for pr in range(NPAIR - 1):
                feeds[pr] = kproj_groups(pr + 1)

            pending = [None]

            def attention(pr, feed):
                """Attention for pair pr; calls next(feed) between steps."""
                for qh in range(2):
                    av = [avps.tile([65, 512], f32, tag="av", name=f"av{hi}")
                          for hi in range(2)]
                    steps = _attn_steps(qh)
                    for si, (kt_idx, peer, lt_min, q0, n) in enumerate(steps):
                        next(feed, None)
                        # scores: both heads concurrent via PE row groups;
                        # head chunks at fixed 512-col (bank) offsets
                        sp = sps.tile([P, 1024], f32, tag="sps")
                        for hi in range(2):
                            nc.tensor.matmul(
                                sp[:, hi * 512:hi * 512 + n],
                                KT[pr][hi * 64:(hi + 1) * 64,
                                       kt_idx * P:(kt_idx + 1) * P],
                                QT[pr][hi * 64:(hi + 1) * 64, q0:q0 + n],
                                start=True, stop=True)
                        ep = ep_pool.tile([P, 1024], bf16, tag="ep")
                        if n == 512:
                            nc.scalar.activation(ep[:, :], sp[:, :], FT.Exp)
                        else:
                            sp3 = sp[:, :].rearrange(
                                "p (hi q) -> p hi q", hi=2)[:, :, 0:n]
                            ep3 = ep[:, :].rearrange(
                                "p (hi q) -> p hi q", hi=2)[:, :, 0:n]
                            nc.scalar.activation(ep3, sp3, FT.Exp)
                        if lt_min * P >= qh * 512:
                            # diagonal block: first 128 query cols of each head
                            for hi in range(2):
                                sl = ep[:, hi * 512:hi * 512 + P]
                                if peer:
                                    nc.vector.tensor_scalar_mul(
                                        sl, sl, pm_sb[:, 0:1])
                                else:
                                    nc.vector.tensor_mul(sl, sl, tril_sb[:, :])
                        first, last = si == 0, si == len(steps) - 1
                        for hi in range(2):
                            nc.tensor.matmul(
                                av[hi][:, q0 - qh * 512:q0 - qh * 512 + n],
                                VO[kt_idx][:, pr * 130 + hi * 65:
                                           pr * 130 + hi * 65 + 65],
                                ep[:, hi * 512:hi * 512 + n],
                                start=first, stop=last)
                    # normalize: O^T[d, q] = av[d, q] / av[64, q]. 1/rowsum =
                    # exp(-ln(rs)) on the scalar engine (Ln and Exp share the
                    # natural_log table set), computed on the partition-64
                    # row, then broadcast to 64 partitions by a K=1 bf16
                    # matmul and staged to SBUF for the DVE multiply
                    next(feed, None)
                    bcs = []
                    for hi in range(2):
                        lnr = rs_pool.tile([65, 512], f32, tag="rs",
                                           name=f"lnr{hi}")
                        nc.scalar.activation(lnr[64:65, :],
                                             av[hi][64:65, :], FT.Ln)
                        rrow = rs_pool.tile([65, 512], bf16, tag="rrow",
                                            name=f"rrow{hi}")
                        nc.scalar.activation(rrow[64:65, :],
                                             lnr[64:65, :], FT.Exp,
                                             scale=-1.0)
                        bc = sps.tile([64, 512], f32, tag="sps",
                                      name=f"bc{hi}")
                        nc.tensor.matmul(bc[:, :], ones_sb[64:65, :],
                                         rrow[64:65, :], start=True, stop=True)
                        bch = rs_pool.tile([64, 512], bf16, tag="bcs",
                                           name=f"bch{hi}")
                        nc.vector.tensor_copy(out=bch[:, :], in_=bc[:, :])
                        bcs.append(bch)
                    # head 0 writes OT rows 0:64 directly; head 1 goes via
                    # an SBUF scratch + DMA partition shift to rows 64:128
                    nc.vector.tensor_mul(OT[pr][0:64, qh * 512:(qh + 1) * 512],
                                         av[0][0:64, :], bcs[0][:, :])
                    osc = osc_pool.tile([64, 512], bf16, tag="osc")
                    nc.vector.tensor_mul(osc[:, :], av[1][0:64, :],
                                         bcs[1][:, :])
                    nc.sync.dma_start(
                        out=OT[pr][64:128, qh * 512:(qh + 1) * 512],
                        in_=osc[:, :])

            for g in prologue:
                g()
            # residual inputs: prefetch during attention
            for it in range(NT):
                nc.sync.dma_start(out=X[it][:, :],
                                  in_=xp.ap()[it * P:(it + 1) * P, :])
            for pr in range(NPAIR):
                fl = iter(feeds[pr])
                attention(pr, (g() for g in fl))
                for g in fl:
                    g()
        ht_es.close()

        # ---------------- Phase C: Wo, residual, LN2, FFN, store --------------
        with tc.tile_pool(name="wos", bufs=1) as wo_pool, \
             tc.tile_pool(name="wops", bufs=2, space="PSUM") as wops:
            wo_sb = wo_pool.tile([P, 8192], bf16, tag="wo")
            nc.sync.dma_start(out=wo_sb[:, :], in_=wo.ap()[:, :])
            for it in range(NT):
                pw = wops.tile([P, 1024], f32, tag="wops")
                for kt in range(NKT):
                    for hf in range(2):
                        nc.tensor.matmul(
                            pw[:, hf * 512:(hf + 1) * 512],
                            OT[kt][:, it * P:(it + 1) * P],
                            wo_sb[:, kt * 1024 + hf * 512:
                                  kt * 1024 + (hf + 1) * 512],
                            start=(kt == 0), stop=(kt == NKT - 1))
                nc.vector.tensor_add(X[it][:, :], pw[:, :], X[it][:, :])
                if flags["bo"]:
                    nc.vector.tensor_add(X[it][:, :], X[it][:, :], bo_sb[:, :])
        ot_es.close()

        with tc.tile_pool(name="ln2", bufs=3) as lnp2, \
             tc.tile_pool(name="z2", bufs=3) as zpool2, \
             tc.tile_pool(name="h2t", bufs=NKT) as h2t_pool, \
             tc.tile_pool(name="tps3", bufs=2, space="PSUM") as tps3, \
             tc.tile_pool(name="ut", bufs=ND) as ut_pool, \
             tc.tile_pool(name="w1s", bufs=8) as w1_pool, \
             tc.tile_pool(name="w2s", bufs=8) as w2_pool, \
             tc.tile_pool(name="ups", bufs=2, space="PSUM") as ups, \
             tc.tile_pool(name="yps", bufs=4, space="PSUM") as yps:
            H2T = [h2t_pool.tile([P, 1024], bf16, tag="h2t", name=f"h2t{i}") for i in range(NKT)]
            for it in range(NT):
                z2 = ln_tile(X[it], lnp2, zpool2)
                transpose_into(z2, H2T, it, tps3)
            for tch in range(2):
                UT = [ut_pool.tile([P, 512], bf16, tag="ut", name=f"ut{i}") for i in range(ND)]
                for d in range(ND):
                    w1_sb = w1_pool.tile([P, 1024], bf16, tag="w1")
                    nc.sync.dma_start(
                        out=w1_sb[:, :],
                        in_=w1.ap()[:, d * 1024:(d + 1) * 1024])
                    pu = ups.tile([P, 512], f32, tag="ups")
                    for kt in range(NKT):
                        nc.tensor.matmul(
                            pu[:, :], w1_sb[:, kt * P:(kt + 1) * P],
                            H2T[kt][:, tch * 512:(tch + 1) * 512],
                            start=(kt == 0), stop=(kt == NKT - 1))
                    # relu(x + b1)
                    nc.vector.tensor_scalar(UT[d][:, :], pu[:, :],
                                            b1_sb[:, d:d + 1], 0.0,
                                            ALU.add, ALU.max)
                for ch in range(2):
                    ypsum = [yps.tile([P, 512], f32, tag="yps", name=f"yps{i}")
                             for i in range(4)]
                    for d in range(ND):
                        w2_sb = w2_pool.tile([P, 512], bf16, tag="w2")
                        nc.sync.dma_start(
                            out=w2_sb[:, :],
                            in_=w2.ap()[:, d * 1024 + ch * 512:
                                        d * 1024 + (ch + 1) * 512])
                        for tt in range(4):
                            nc.tensor.matmul(
                                ypsum[tt][:, :],
                                UT[d][:, tt * P:(tt + 1) * P],
                                w2_sb[:, :],
                                start=(d == 0), stop=(d == ND - 1))
                    for tt in range(4):
                        it = tch * 4 + tt
                        xsl = X[it][:, ch * 512:(ch + 1) * 512]
                        nc.vector.tensor_add(xsl, ypsum[tt][:, :], xsl)
                        if flags["b2"]:
                            nc.vector.tensor_add(
                                xsl, xsl, b2_sb[:, ch * 512:(ch + 1) * 512])
            for it in range(NT):
                nc.sync.dma_start(out=out.ap()[it * P:(it + 1) * P, :],
                                  in_=X[it][:, :])

    nc.compile()
    return nc


_CACHE = {}


def _prep(inputs):
    """Host-side preprocessing: fold LN affine into weights, tile/cast, shard."""
    x = np.asarray(inputs["x"], np.float32)
    Wq = np.asarray(inputs["Wq"], np.float32)
    Wk = np.asarray(inputs["Wk"], np.float32)
    Wv = np.asarray(inputs["Wv"], np.float32)
    Wo = np.asarray(inputs["Wo"], np.float32)
    bo = np.asarray(inputs["bo"], np.float32)
    W1 = np.asarray(inputs["W1"], np.float32)
    b1 = np.asarray(inputs["b1"], np.float32)
    W2 = np.asarray(inputs["W2"], np.float32)
    b2 = np.asarray(inputs["b2"], np.float32)
    g1 = np.asarray(inputs["g1"], np.float32)
    be1 = np.asarray(inputs["be1"], np.float32)
    g2 = np.asarray(inputs["g2"], np.float32)
    be2 = np.asarray(inputs["be2"], np.float32)

    Wq_g = (Wq * g1[None, :, None]).astype(BF16)   # [16,1024,64]
    Wk_g = (Wk * g1[None, :, None]).astype(BF16)
    Wv_g = (Wv * g1[None, :, None]).astype(BF16)
    qb = np.einsum('c,hcd->hd', be1, Wq_g.astype(np.float32))  # [16,64]
    kb = np.einsum('c,hcd->hd', be1, Wk_g.astype(np.float32))
    vb = np.einsum('c,hcd->hd', be1, Wv_g.astype(np.float32))
    if np.abs(vb).max() > 0:
        raise NotImplementedError("nonzero folded V bias not supported")

    def lhsT_pack(wflat):  # [1024 c, 1024 m] -> [128, (pair, kt, 128)]
        return np.ascontiguousarray(
            wflat.reshape(8, 128, 8, 128).transpose(1, 2, 0, 3).reshape(128, 8192))

    def rhs_pack(wflat):   # [1024 k, 1024 n] -> [128, (kt, 1024)]
        return np.ascontiguousarray(
            wflat.reshape(8, 128, 1024).transpose(1, 0, 2).reshape(128, 8192))

    wq_h = lhsT_pack(Wq_g.transpose(1, 0, 2).reshape(1024, 1024))
    wk_h = lhsT_pack(Wk_g.transpose(1, 0, 2).reshape(1024, 1024))
    wv_h = rhs_pack(Wv_g.transpose(1, 0, 2).reshape(1024, 1024))
    wo_h = rhs_pack(Wo.astype(BF16))
    W1_g = (W1 * g2[:, None]).astype(BF16)         # [1024, 4096]
    b1p = b1 + be2 @ W1_g.astype(np.float32)
    w1_h = np.ascontiguousarray(
        W1_g.reshape(8, 128, 32, 128).transpose(1, 2, 0, 3).reshape(128, 32768))
    w2_h = np.ascontiguousarray(
        W2.astype(BF16).reshape(32, 128, 1024).transpose(1, 0, 2).reshape(128, 32768))

    # per-pair stacked [128, 8] bias tables
    qb_t = np.zeros((128, 8), np.float32)
    kb_t = np.zeros((128, 8), np.float32)
    for pr in range(8):
        qb_t[0:64, pr] = qb[2 * pr]
        qb_t[64:128, pr] = qb[2 * pr + 1]
        kb_t[0:64, pr] = kb[2 * pr]
        kb_t[64:128, pr] = kb[2 * pr + 1]
    b1_t = np.ascontiguousarray(b1p.reshape(32, 128).T.astype(np.float32))
    bo_t = np.broadcast_to(bo, (128, 1024)).astype(np.float32).copy()
    b2_t = np.broadcast_to(b2, (128, 1024)).astype(np.float32).copy()

    tril = np.triu(np.ones((128, 128), np.float32)).astype(BF16)
    ident = np.eye(128, dtype=np.float32).astype(BF16)

    flags = {"bo": bool(np.abs(bo).max() > 0), "b2": bool(np.abs(b2).max() > 0)}

    shared = dict(wq=wq_h, wk=wk_h, wv=wv_h, wo=wo_h, w1=w1_h, w2=w2_h,
                  trilq=tril, identd=ident, qbias=qb_t, kbias=kb_t,
                  b1p=b1_t, bo_row=bo_t, b2_row=b2_t)
    in_maps = []
    xt = x.reshape(4, 16, 128, 1024)
    for core in range(8):
        b, p = core // 2, core % 2
        own = xt[b, p::2]                  # [8, 128, 1024] global tiles 2lt+p
        peer = xt[b, 1 - p::2]             # [8, 128, 1024] global tiles 2m+1-p
        xperm = np.concatenate([own, peer], 0).reshape(2048, 1024)
        pm = np.full((128, 1), 1.0 if p == 1 else 0.0, np.float32)
        in_maps.append({"xp": np.ascontiguousarray(xperm), "pmask": pm,
                        **shared})
    return in_maps, flags


def _get_nc(flags):
    key = tuple(sorted(flags.items()))
    if key not in _CACHE:
        _CACHE[key] = _build(flags)
    return _CACHE[key]


def run(inputs, **kw):
    in_maps, flags = _prep(inputs)
    nc = _get_nc(flags)
    res = run_bass_kernel_spmd(nc, in_maps, core_ids=list(range(8)), **kw)
    x = np.asarray(inputs["x"], np.float32)
    outf = np.zeros_like(x)
    for core in range(8):
        b, p = core // 2, core % 2
        o = res.results[core]["out"].reshape(8, 128, 1024)
        outf[b].reshape(16, 128, 1024)[p::2] = o
    return outf, res


def kernel(**inputs):
    outf, _ = run(inputs)
    return outf
